# revision 16
# baseline (speedup 1.0000x reference)
"""OTAM (5-way 5-shot video few-shot) kernel for Trainium2, 8 NeuronCores.

Self-contained: kernel(**inputs) takes full inputs, shards 512 queries over
8 cores (64 each), runs a Bass/Tile kernel per core, gathers class means.

v4 design (v0 328us, v1 175us, v2 142us, v3 131us):
 - bf16 matmuls + transposes; norms ACT Square + Quake rsqrt (DVE); q stays
   unnormalized, 10/||q|| folded into the exp scale AP (norm branch runs
   parallel to the transpose branch)
 - support tensor reordered HOST-side to (tau, s) row order so the staged
   cost tile is [t][tau][s] -> every DP operand is contiguous
 - software-pipelined PE stream: transposes of group g+1 are emitted before
   the matmuls of group g so the PE never waits on the scalar qt copy
 - cost tile staged twice (both DP dirs) via 800B-packet DMAs; DP on 128
   partitions, W layout [l][s], bf16, 2 renorms (int16 exponent tricks)
"""
import sys
sys.path.insert(0, "/opt/trn_rl_repo")
import numpy as np
from contextlib import ExitStack

import concourse.bacc as bacc
import concourse.tile as tile
from concourse import mybir
from concourse.masks import make_identity

F32 = mybir.dt.float32
BF16 = mybir.dt.bfloat16
I32 = mybir.dt.int32
I16 = mybir.dt.int16
AF = mybir.ActivationFunctionType
ALU = mybir.AluOpType
LN2 = float(np.log(2.0))

NS, T, D = 25, 16, 2048
NQ_CORE = 64
G = 8                        # query groups of 128 rows (8 queries) each
NSTAU = NS * T               # 400
KCH = D // 128               # 16
SROWS = [128, 128, 128, 16]


def quake_rsqrt(nc, pool, x_f32, nrow, tag, scale=1.0):
    """y ~= scale/sqrt(x) on [nrow,1] f32 (DVE only; 1 Newton iteration)."""
    y = pool.tile([128, 1], F32, tag=tag + "_y")
    t = pool.tile([128, 1], F32, tag=tag + "_t")
    yi = y.bitcast(I32)
    nc.vector.tensor_scalar(yi[:nrow], x_f32[:nrow].bitcast(I32), 1, None,
                            op0=ALU.logical_shift_right)
    nc.vector.tensor_scalar(yi[:nrow], yi[:nrow], 0x5F3759DF, -1,
                            op0=ALU.subtract, op1=ALU.mult)
    nc.vector.tensor_tensor(t[:nrow], y[:nrow], y[:nrow], op=ALU.mult)
    nc.vector.tensor_tensor(t[:nrow], t[:nrow], x_f32[:nrow], op=ALU.mult)
    nc.vector.tensor_scalar(t[:nrow], t[:nrow], -0.5 * scale, 1.5 * scale,
                            op0=ALU.mult, op1=ALU.add)
    nc.vector.tensor_tensor(y[:nrow], y[:nrow], t[:nrow], op=ALU.mult)
    return y


def build_core_kernel():
    nc = bacc.Bacc("TRN2", target_bir_lowering=False, debug=False)

    q_d = nc.dram_tensor("q", [NQ_CORE * T, D], F32, kind="ExternalInput").ap()
    # s rows are HOST-reordered to (tau, s): row index = tau*NS + s
    s_d = nc.dram_tensor("s", [NSTAU, D], F32, kind="ExternalInput").ap()
    out_d = nc.dram_tensor("out", [128, NS], F32, kind="ExternalOutput").ap()

    with tile.TileContext(nc) as tc, ExitStack() as ctx:
        const = ctx.enter_context(tc.tile_pool(name="const", bufs=1))
        eye_b = const.tile([128, 128], BF16, tag="eye_b")
        make_identity(nc, eye_b[:])
        bias_m10 = const.tile([128, 1], F32, tag="bias_m10")
        nc.vector.memset(bias_m10[:], -10.0)

        # normalized+transposed support: st_b[p=d%128][k=d//128][col=(tau,s)]
        stp = ctx.enter_context(tc.tile_pool(name="stp", bufs=1))
        st_b = stp.tile([128, KCH, NSTAU], BF16, tag="st_b")

        nsc = ctx.enter_context(tc.tile_pool(name="nsc", bufs=1))
        dmp = ctx.enter_context(tc.tile_pool(name="dmp", bufs=1))
        dump = dmp.tile([128, D], BF16, tag="dump")

        # ---------------- pools ----------------
        cp = ctx.enter_context(tc.tile_pool(name="cp", bufs=1))
        # c_t[p][t][tau][s] bf16: partitions q and 64+q hold query q's costs
        c_t = cp.tile([128, T, T, NS], BF16, tag="c_t")

        qldp = ctx.enter_context(tc.tile_pool(name="qldp", bufs=3))
        qbfp = ctx.enter_context(tc.tile_pool(name="qbfp", bufs=3))
        qtp = ctx.enter_context(tc.tile_pool(name="qtp", bufs=3))
        t1p = ctx.enter_context(tc.tile_pool(name="t1p", bufs=2))
        ptr = ctx.enter_context(tc.tile_pool(name="ptr", bufs=2, space="PSUM"))
        pmm = ctx.enter_context(tc.tile_pool(name="pmm", bufs=2, space="PSUM"))

        # ---------------- Q phase: software-pipelined over 8 groups -------
        def q_load(g):
            qraw = qldp.tile([128, D], F32, tag="qraw")
            nc.sync.dma_start(out=qraw[:], in_=q_d[128 * g:128 * (g + 1), :])
            return qraw

        def q_norm(qbf, g):
            n2 = nsc.tile([128, 1], F32, tag="qn2")
            if g % 2 == 0:
                nc.scalar.activation(dump[:], qbf[:], AF.Square,
                                     accum_out=n2[:])
            else:
                nc.vector.tensor_tensor(dump[:], qbf[:], qbf[:], op=ALU.mult)
                nc.vector.tensor_reduce(n2[:], dump[:],
                                        axis=mybir.AxisListType.X, op=ALU.add)
            return quake_rsqrt(nc, nsc, n2, 128, "qrs", scale=10.0)

        def q_cast(qraw):
            qbf = qbfp.tile([128, D], BF16, tag="qbf")
            nc.vector.tensor_copy(qbf[:], qraw[:])
            return qbf

        def q_transpose(qbf):
            pt = ptr.tile([128, D], BF16, tag="pt")
            for k in range(KCH):
                nc.tensor.transpose(pt[:, 128 * k:128 * (k + 1)],
                                    qbf[:, 128 * k:128 * (k + 1)], eye_b[:])
            qt = qtp.tile([128, KCH, 128], BF16, tag="qt")
            half = KCH // 2
            ptv = pt[:].rearrange("p (k f) -> p k f", k=KCH)
            nc.scalar.copy(qt[:, 0:half], ptv[:, 0:half])
            nc.vector.tensor_copy(qt[:, half:KCH], ptv[:, half:KCH])
            return qt

        def q_mm_exp_stage(g, qt, rq10):
            mm = pmm.tile([128, NSTAU], F32, tag="mm")
            for k in range(KCH):
                nc.tensor.matmul(mm[:], qt[:, k, :], st_b[:, k, :],
                                 start=(k == 0), stop=(k == KCH - 1))
            t1 = t1p.tile([128, NSTAU], BF16, tag="t1")
            nc.scalar.activation(t1[:], mm[:], AF.Exp, bias=bias_m10[:],
                                 scale=rq10[:])
            nc.scalar.dma_start(out=c_t[8 * g:8 * (g + 1), :, :, :], in_=t1[:])
            nc.sync.dma_start(out=c_t[64 + 8 * g:64 + 8 * (g + 1), :, :, :],
                              in_=t1[:])


        # ---------------- fronts for groups 0 and 1 (before S phase) ----
        qraws = [None] * G
        qts = [None] * G
        rqs = [None] * G
        qraws[0] = q_load(0)
        qraws[1] = q_load(1)

        # ---------------- S phase ----------------
        with tc.tile_pool(name="sraw", bufs=1) as sraw, \
             tc.tile_pool(name="spsum", bufs=2, space="PSUM") as spsum:
            snorm = []
            for i, nrow in enumerate(SROWS):
                t_ = sraw.tile([128, D], F32, tag=f"sraw{i}")
                nc.sync.dma_start(out=t_[:nrow],
                                  in_=s_d[128 * i:128 * i + nrow, :])
                sb = sraw.tile([128, D], BF16, tag=f"sbf{i}")
                nc.vector.tensor_copy(sb[:nrow], t_[:nrow])
                snorm.append((sb, nrow))
            n2s = []
            for i, (sb, nrow) in enumerate(snorm):
                n2 = nsc.tile([128, 1], F32, tag=f"sn2_{i}")
                nc.scalar.activation(dump[:nrow], sb[:nrow], AF.Square,
                                     accum_out=n2[:nrow])
                n2s.append(n2)
            for i, (sb, nrow) in enumerate(snorm):
                rs = quake_rsqrt(nc, nsc, n2s[i], nrow, f"srs{i}")
                nc.vector.tensor_scalar(sb[:nrow], sb[:nrow], rs[:nrow], None,
                                        op0=ALU.mult)
            for k in range(KCH):
                ps = spsum.tile([128, 512], BF16, tag="sps")
                for i, (sb, nrow) in enumerate(snorm):
                    nc.tensor.transpose(ps[:, 128 * i:128 * i + nrow],
                                        sb[:nrow, 128 * k:128 * (k + 1)],
                                        eye_b[:nrow, :nrow])
                nc.scalar.copy(st_b[:, k, :], ps[:, 0:NSTAU])

        qbf0 = q_cast(qraws[0])
        rqs[0] = q_norm(qbf0, 0)
        qts[0] = q_transpose(qbf0)
        qbf1 = q_cast(qraws[1])
        rqs[1] = q_norm(qbf1, 1)
        qts[1] = q_transpose(qbf1)

        # modulo software pipeline with distance 2: fronts for groups 0/1
        # were already emitted before the S phase; the loop emits front(g+2)
        # then back(g).
        for g in range(G):
            if g + 2 < G:
                qraws[g + 2] = q_load(g + 2)
                qbf_n = q_cast(qraws[g + 2])
                rqs[g + 2] = q_norm(qbf_n, g + 2)
                qts[g + 2] = q_transpose(qbf_n)
            q_mm_exp_stage(g, qts[g], rqs[g])

        # ---------------- DP phase (exp domain) ----------------
        # partition q: dir "b" (rows l = support frame tau, cols = t)
        # partition 64+q: dir "a" (rows l = query frame t, cols = tau)
        # W layout [l][s] so W slices and dir-b cost reads are contiguous
        dpp = ctx.enter_context(tc.tile_pool(name="dpp", bufs=1))
        w_t = dpp.tile([128, T + 1, NS], BF16, tag="w_t")
        nc.vector.memset(w_t[:], 2.0)
        nc.vector.memset(w_t[:, 0:1, :], 1.0)
        o_t = dpp.tile([128, NS], F32, tag="o_t")
        nc.vector.memset(o_t[:], 0.0)
        scratch = dpp.tile([128, T, NS], BF16, tag="scratch")
        kmax = dpp.tile([128, NS], BF16, tag="kmax")
        masked = dpp.tile([128, NS], I16, tag="masked")
        krec = dpp.tile([128, NS], I16, tag="krec")
        ef = dpp.tile([128, NS], F32, tag="ef")
        otmp = dpp.tile([128, NS], F32, tag="otmp")

        def renorm(a):
            wsl = w_t[:, a:T + 1, :]
            nc.vector.tensor_reduce(kmax[:], wsl.rearrange("p l s -> p s l"),
                                    axis=mybir.AxisListType.X, op=ALU.max)
            nc.vector.tensor_scalar(masked[:], kmax[:].bitcast(I16),
                                    0x7F80, None, op0=ALU.bitwise_and)
            nc.vector.tensor_scalar(krec[:], masked[:], 0x7F00, -1,
                                    op0=ALU.subtract, op1=ALU.mult)
            nc.vector.tensor_copy(ef[:], masked[:])
            nc.vector.tensor_scalar(otmp[:], ef[:], LN2 / (1 << 7),
                                    -127.0 * LN2, op0=ALU.mult, op1=ALU.add)
            nc.vector.tensor_tensor(o_t[:], o_t[:], otmp[:], op=ALU.add)
            nc.vector.tensor_tensor(
                wsl, wsl,
                krec[:].bitcast(BF16).unsqueeze(-1)
                    .broadcast_to((128, NS, T + 1 - a))
                    .rearrange("p s l -> p l s"),
                op=ALU.mult)

        for m in range(2, T + 3):           # m = 2..18
            j0 = max(1, m - 2)
            wm = (T + 1) - j0
            if m == T + 2:                  # last: dup, cost=1, l=T only
                nc.vector.scalar_tensor_tensor(
                    w_t[:, T:T + 1, :], w_t[:, T:T + 1, :], 2.0,
                    w_t[:, T - 1:T, :], op0=ALU.mult, op1=ALU.add)
                break
            wact = w_t[:, j0:T + 1, :]
            wsh = w_t[:, j0 - 1:T, :]
            tmp = scratch[:, 0:wm, :]
            if m == 2:
                nc.vector.scalar_tensor_tensor(tmp, wact, 2.0, wsh,
                                               op0=ALU.mult, op1=ALU.add)
            else:
                nc.vector.tensor_tensor(tmp, wact, wsh, op=ALU.add)
            # dir b on partitions 0:64 (contiguous cost read)
            cb = c_t[0:64, m - 2, j0 - 1:j0 - 1 + wm, :]
            nc.vector.tensor_tensor(w_t[0:64, j0:T + 1, :], tmp[0:64], cb,
                                    op=ALU.mult)
            # dir a on partitions 64:128 (l strided, s contiguous)
            ca = c_t[64:128, j0 - 1:j0 - 1 + wm, m - 2, :]
            nc.vector.tensor_tensor(w_t[64:128, j0:T + 1, :], tmp[64:128], ca,
                                    op=ALU.mult)
            if m in (9, 16):
                renorm(m - 2)

        lnw = dpp.tile([128, NS], F32, tag="lnw")
        nc.scalar.activation(lnw[:], w_t[:, T, :], AF.Ln)
        ans = dpp.tile([128, NS], F32, tag="ans")
        nc.vector.tensor_tensor(ans[:], lnw[:], o_t[:], op=ALU.add)
        nc.sync.dma_start(out=out_d, in_=ans[:])

    nc.compile()
    return nc


_NC_CACHE = {}


def _get_nc():
    if "nc" not in _NC_CACHE:
        _NC_CACHE["nc"] = build_core_kernel()
    return _NC_CACHE["nc"]


def kernel(support_features, target_features, support_labels):
    out, _ = host_kernel(support_features, target_features, support_labels,
                         nc=_get_nc())
    return out


def host_kernel(support_features, target_features, support_labels, nc=None,
                run_hw=True, trace=False):
    n_support, T_, d = support_features.shape
    nq = target_features.shape[0]
    assert (n_support, T_, d) == (NS, T, D) and nq == 512
    if nc is None:
        nc = build_core_kernel()
    # reorder support rows to (tau, s) so mm columns come out (tau, s)-major
    s_flat = np.ascontiguousarray(
        np.asarray(support_features).transpose(1, 0, 2).reshape(NSTAU, D))
    in_maps = []
    for c in range(8):
        qs = target_features[64 * c:64 * (c + 1)].reshape(NQ_CORE * T, D)
        in_maps.append({"q": np.ascontiguousarray(qs), "s": s_flat})
    from concourse.bass_utils import run_bass_kernel_spmd
    res = run_bass_kernel_spmd(nc, in_maps, list(range(8)), trace=trace)
    outs = [np.asarray(r["out"]) for r in res.results]
    # partition q = dir b, partition 64+q = dir a; dists = -0.1*(a+b)
    dists = np.concatenate(
        [-0.1 * (o[0:64] + o[64:128]) for o in outs], axis=0)
    onehot = (np.asarray(support_labels)[:, None]
              == np.arange(5)[None, :]).astype(np.float32)
    class_dists = (dists.astype(np.float32) @ onehot) / onehot.sum(axis=0)
    return class_dists.astype(np.float32), res


# revision 17
# speedup vs baseline: 1.0034x; 1.0034x over previous
"""OTAM (5-way 5-shot video few-shot) kernel for Trainium2, 8 NeuronCores.

Self-contained: kernel(**inputs) takes full inputs, shards 512 queries over
8 cores (64 each), runs a Bass/Tile kernel per core, gathers class means.

v4 design (v0 328us, v1 175us, v2 142us, v3 131us):
 - bf16 matmuls + transposes; norms ACT Square + Quake rsqrt (DVE); q stays
   unnormalized, 10/||q|| folded into the exp scale AP (norm branch runs
   parallel to the transpose branch)
 - support tensor reordered HOST-side to (tau, s) row order so the staged
   cost tile is [t][tau][s] -> every DP operand is contiguous
 - software-pipelined PE stream: transposes of group g+1 are emitted before
   the matmuls of group g so the PE never waits on the scalar qt copy
 - cost tile staged twice (both DP dirs) via 800B-packet DMAs; DP on 128
   partitions, W layout [l][s], bf16, 2 renorms (int16 exponent tricks)
"""
import sys
sys.path.insert(0, "/opt/trn_rl_repo")
import numpy as np
from contextlib import ExitStack

import concourse.bacc as bacc
import concourse.tile as tile
from concourse import mybir
from concourse.masks import make_identity

F32 = mybir.dt.float32
BF16 = mybir.dt.bfloat16
I32 = mybir.dt.int32
I16 = mybir.dt.int16
AF = mybir.ActivationFunctionType
ALU = mybir.AluOpType
LN2 = float(np.log(2.0))

NS, T, D = 25, 16, 2048
NQ_CORE = 64
G = 8                        # query groups of 128 rows (8 queries) each
NSTAU = NS * T               # 400
KCH = D // 128               # 16
SROWS = [128, 128, 128, 16]


def quake_rsqrt(nc, pool, x_f32, nrow, tag, scale=1.0):
    """y ~= scale/sqrt(x) on [nrow,1] f32 (DVE only; 1 Newton iteration)."""
    y = pool.tile([128, 1], F32, tag=tag + "_y")
    t = pool.tile([128, 1], F32, tag=tag + "_t")
    yi = y.bitcast(I32)
    nc.vector.tensor_scalar(yi[:nrow], x_f32[:nrow].bitcast(I32), 1, None,
                            op0=ALU.logical_shift_right)
    nc.vector.tensor_scalar(yi[:nrow], yi[:nrow], 0x5F3759DF, -1,
                            op0=ALU.subtract, op1=ALU.mult)
    nc.vector.tensor_tensor(t[:nrow], y[:nrow], y[:nrow], op=ALU.mult)
    nc.vector.tensor_tensor(t[:nrow], t[:nrow], x_f32[:nrow], op=ALU.mult)
    nc.vector.tensor_scalar(t[:nrow], t[:nrow], -0.5 * scale, 1.5 * scale,
                            op0=ALU.mult, op1=ALU.add)
    nc.vector.tensor_tensor(y[:nrow], y[:nrow], t[:nrow], op=ALU.mult)
    return y


def build_core_kernel():
    nc = bacc.Bacc("TRN2", target_bir_lowering=False, debug=False)

    q_d = nc.dram_tensor("q", [NQ_CORE * T, D], F32, kind="ExternalInput").ap()
    # s rows are HOST-reordered to (tau, s): row index = tau*NS + s
    s_d = nc.dram_tensor("s", [NSTAU, D], F32, kind="ExternalInput").ap()
    out_d = nc.dram_tensor("out", [128, NS], F32, kind="ExternalOutput").ap()

    with tile.TileContext(nc) as tc, ExitStack() as ctx:
        const = ctx.enter_context(tc.tile_pool(name="const", bufs=1))
        eye_b = const.tile([128, 128], BF16, tag="eye_b")
        make_identity(nc, eye_b[:])
        bias_m10 = const.tile([128, 1], F32, tag="bias_m10")
        nc.vector.memset(bias_m10[:], -10.0)

        # normalized+transposed support: st_b[p=d%128][k=d//128][col=(tau,s)]
        stp = ctx.enter_context(tc.tile_pool(name="stp", bufs=1))
        st_b = stp.tile([128, KCH, NSTAU], BF16, tag="st_b")

        nsc = ctx.enter_context(tc.tile_pool(name="nsc", bufs=1))
        dmp = ctx.enter_context(tc.tile_pool(name="dmp", bufs=1))
        dump = dmp.tile([128, D], BF16, tag="dump")

        # ---------------- pools ----------------
        cp = ctx.enter_context(tc.tile_pool(name="cp", bufs=1))
        # c_t[p][t][tau][s] bf16: partitions q and 64+q hold query q's costs
        c_t = cp.tile([128, T, T, NS], BF16, tag="c_t")

        qldp = ctx.enter_context(tc.tile_pool(name="qldp", bufs=3))
        qbfp = ctx.enter_context(tc.tile_pool(name="qbfp", bufs=3))
        qtp = ctx.enter_context(tc.tile_pool(name="qtp", bufs=3))
        t1p = ctx.enter_context(tc.tile_pool(name="t1p", bufs=2))
        ptr = ctx.enter_context(tc.tile_pool(name="ptr", bufs=2, space="PSUM"))
        pmm = ctx.enter_context(tc.tile_pool(name="pmm", bufs=2, space="PSUM"))

        # ---------------- Q phase: software-pipelined over 8 groups -------
        def q_load(g):
            qraw = qldp.tile([128, D], F32, tag="qraw")
            nc.sync.dma_start(out=qraw[:], in_=q_d[128 * g:128 * (g + 1), :])
            return qraw

        def q_norm(qbf, g):
            n2 = nsc.tile([128, 1], F32, tag="qn2")
            nc.scalar.activation(dump[:], qbf[:], AF.Square,
                                 accum_out=n2[:])
            return quake_rsqrt(nc, nsc, n2, 128, "qrs", scale=10.0)

        def q_cast(qraw):
            qbf = qbfp.tile([128, D], BF16, tag="qbf")
            nc.vector.tensor_copy(qbf[:], qraw[:])
            return qbf

        def q_transpose(qbf):
            pt = ptr.tile([128, D], BF16, tag="pt")
            for k in range(KCH):
                nc.tensor.transpose(pt[:, 128 * k:128 * (k + 1)],
                                    qbf[:, 128 * k:128 * (k + 1)], eye_b[:])
            qt = qtp.tile([128, KCH, 128], BF16, tag="qt")
            half = KCH // 2
            ptv = pt[:].rearrange("p (k f) -> p k f", k=KCH)
            nc.scalar.copy(qt[:, 0:half], ptv[:, 0:half])
            nc.vector.tensor_copy(qt[:, half:KCH], ptv[:, half:KCH])
            return qt

        def q_mm_exp_stage(g, qt, rq10):
            mm = pmm.tile([128, NSTAU], F32, tag="mm")
            for k in range(KCH):
                nc.tensor.matmul(mm[:], qt[:, k, :], st_b[:, k, :],
                                 start=(k == 0), stop=(k == KCH - 1))
            t1 = t1p.tile([128, NSTAU], BF16, tag="t1")
            nc.scalar.activation(t1[:], mm[:], AF.Exp, bias=bias_m10[:],
                                 scale=rq10[:])
            nc.scalar.dma_start(out=c_t[8 * g:8 * (g + 1), :, :, :], in_=t1[:])
            nc.sync.dma_start(out=c_t[64 + 8 * g:64 + 8 * (g + 1), :, :, :],
                              in_=t1[:])


        # ---------------- fronts for groups 0 and 1 (before S phase) ----
        qraws = [None] * G
        qts = [None] * G
        rqs = [None] * G
        qraws[0] = q_load(0)
        qraws[1] = q_load(1)

        # ---------------- S phase ----------------
        with tc.tile_pool(name="sraw", bufs=1) as sraw, \
             tc.tile_pool(name="spsum", bufs=2, space="PSUM") as spsum:
            snorm = []
            for i, nrow in enumerate(SROWS):
                t_ = sraw.tile([128, D], F32, tag=f"sraw{i}")
                nc.sync.dma_start(out=t_[:nrow],
                                  in_=s_d[128 * i:128 * i + nrow, :])
                sb = sraw.tile([128, D], BF16, tag=f"sbf{i}")
                nc.vector.tensor_copy(sb[:nrow], t_[:nrow])
                snorm.append((sb, nrow))
            n2s = []
            for i, (sb, nrow) in enumerate(snorm):
                n2 = nsc.tile([128, 1], F32, tag=f"sn2_{i}")
                nc.scalar.activation(dump[:nrow], sb[:nrow], AF.Square,
                                     accum_out=n2[:nrow])
                n2s.append(n2)
            for i, (sb, nrow) in enumerate(snorm):
                rs = quake_rsqrt(nc, nsc, n2s[i], nrow, f"srs{i}")
                nc.vector.tensor_scalar(sb[:nrow], sb[:nrow], rs[:nrow], None,
                                        op0=ALU.mult)
            for k in range(KCH):
                ps = spsum.tile([128, 512], BF16, tag="sps")
                for i, (sb, nrow) in enumerate(snorm):
                    nc.tensor.transpose(ps[:, 128 * i:128 * i + nrow],
                                        sb[:nrow, 128 * k:128 * (k + 1)],
                                        eye_b[:nrow, :nrow])
                nc.scalar.copy(st_b[:, k, :], ps[:, 0:NSTAU])

        qbf0 = q_cast(qraws[0])
        rqs[0] = q_norm(qbf0, 0)
        qts[0] = q_transpose(qbf0)
        qbf1 = q_cast(qraws[1])
        rqs[1] = q_norm(qbf1, 1)
        qts[1] = q_transpose(qbf1)

        # modulo software pipeline with distance 2: fronts for groups 0/1
        # were already emitted before the S phase; the loop emits front(g+2)
        # then back(g).
        for g in range(G):
            if g + 2 < G:
                qraws[g + 2] = q_load(g + 2)
                qbf_n = q_cast(qraws[g + 2])
                rqs[g + 2] = q_norm(qbf_n, g + 2)
                qts[g + 2] = q_transpose(qbf_n)
            q_mm_exp_stage(g, qts[g], rqs[g])

        # ---------------- DP phase (exp domain) ----------------
        # partition q: dir "b" (rows l = support frame tau, cols = t)
        # partition 64+q: dir "a" (rows l = query frame t, cols = tau)
        # W layout [l][s] so W slices and dir-b cost reads are contiguous
        dpp = ctx.enter_context(tc.tile_pool(name="dpp", bufs=1))
        w_t = dpp.tile([128, T + 1, NS], BF16, tag="w_t")
        nc.vector.memset(w_t[:], 2.0)
        nc.vector.memset(w_t[:, 0:1, :], 1.0)
        o_t = dpp.tile([128, NS], F32, tag="o_t")
        nc.vector.memset(o_t[:], 0.0)
        scratch = dpp.tile([128, T, NS], BF16, tag="scratch")
        kmax = dpp.tile([128, NS], BF16, tag="kmax")
        masked = dpp.tile([128, NS], I16, tag="masked")
        krec = dpp.tile([128, NS], I16, tag="krec")
        ef = dpp.tile([128, NS], F32, tag="ef")
        otmp = dpp.tile([128, NS], F32, tag="otmp")

        def renorm(a):
            wsl = w_t[:, a:T + 1, :]
            nc.vector.tensor_reduce(kmax[:], wsl.rearrange("p l s -> p s l"),
                                    axis=mybir.AxisListType.X, op=ALU.max)
            nc.vector.tensor_scalar(masked[:], kmax[:].bitcast(I16),
                                    0x7F80, None, op0=ALU.bitwise_and)
            nc.vector.tensor_scalar(krec[:], masked[:], 0x7F00, -1,
                                    op0=ALU.subtract, op1=ALU.mult)
            nc.vector.tensor_copy(ef[:], masked[:])
            nc.vector.tensor_scalar(otmp[:], ef[:], LN2 / (1 << 7),
                                    -127.0 * LN2, op0=ALU.mult, op1=ALU.add)
            nc.vector.tensor_tensor(o_t[:], o_t[:], otmp[:], op=ALU.add)
            nc.vector.tensor_tensor(
                wsl, wsl,
                krec[:].bitcast(BF16).unsqueeze(-1)
                    .broadcast_to((128, NS, T + 1 - a))
                    .rearrange("p s l -> p l s"),
                op=ALU.mult)

        for m in range(2, T + 3):           # m = 2..18
            j0 = max(1, m - 2)
            wm = (T + 1) - j0
            if m == T + 2:                  # last: dup, cost=1, l=T only
                nc.vector.scalar_tensor_tensor(
                    w_t[:, T:T + 1, :], w_t[:, T:T + 1, :], 2.0,
                    w_t[:, T - 1:T, :], op0=ALU.mult, op1=ALU.add)
                break
            wact = w_t[:, j0:T + 1, :]
            wsh = w_t[:, j0 - 1:T, :]
            tmp = scratch[:, 0:wm, :]
            if m == 2:
                nc.vector.scalar_tensor_tensor(tmp, wact, 2.0, wsh,
                                               op0=ALU.mult, op1=ALU.add)
            else:
                nc.vector.tensor_tensor(tmp, wact, wsh, op=ALU.add)
            # dir b on partitions 0:64 (contiguous cost read)
            cb = c_t[0:64, m - 2, j0 - 1:j0 - 1 + wm, :]
            nc.vector.tensor_tensor(w_t[0:64, j0:T + 1, :], tmp[0:64], cb,
                                    op=ALU.mult)
            # dir a on partitions 64:128 (l strided, s contiguous)
            ca = c_t[64:128, j0 - 1:j0 - 1 + wm, m - 2, :]
            nc.vector.tensor_tensor(w_t[64:128, j0:T + 1, :], tmp[64:128], ca,
                                    op=ALU.mult)
            if m in (9, 16):
                renorm(m - 2)

        lnw = dpp.tile([128, NS], F32, tag="lnw")
        nc.scalar.activation(lnw[:], w_t[:, T, :], AF.Ln)
        ans = dpp.tile([128, NS], F32, tag="ans")
        nc.vector.tensor_tensor(ans[:], lnw[:], o_t[:], op=ALU.add)
        nc.sync.dma_start(out=out_d, in_=ans[:])

    nc.compile()
    return nc


_NC_CACHE = {}


def _get_nc():
    if "nc" not in _NC_CACHE:
        _NC_CACHE["nc"] = build_core_kernel()
    return _NC_CACHE["nc"]


def kernel(support_features, target_features, support_labels):
    out, _ = host_kernel(support_features, target_features, support_labels,
                         nc=_get_nc())
    return out


def host_kernel(support_features, target_features, support_labels, nc=None,
                run_hw=True, trace=False):
    n_support, T_, d = support_features.shape
    nq = target_features.shape[0]
    assert (n_support, T_, d) == (NS, T, D) and nq == 512
    if nc is None:
        nc = build_core_kernel()
    # reorder support rows to (tau, s) so mm columns come out (tau, s)-major
    s_flat = np.ascontiguousarray(
        np.asarray(support_features).transpose(1, 0, 2).reshape(NSTAU, D))
    in_maps = []
    for c in range(8):
        qs = target_features[64 * c:64 * (c + 1)].reshape(NQ_CORE * T, D)
        in_maps.append({"q": np.ascontiguousarray(qs), "s": s_flat})
    from concourse.bass_utils import run_bass_kernel_spmd
    res = run_bass_kernel_spmd(nc, in_maps, list(range(8)), trace=trace)
    outs = [np.asarray(r["out"]) for r in res.results]
    # partition q = dir b, partition 64+q = dir a; dists = -0.1*(a+b)
    dists = np.concatenate(
        [-0.1 * (o[0:64] + o[64:128]) for o in outs], axis=0)
    onehot = (np.asarray(support_labels)[:, None]
              == np.arange(5)[None, :]).astype(np.float32)
    class_dists = (dists.astype(np.float32) @ onehot) / onehot.sum(axis=0)
    return class_dists.astype(np.float32), res


# revision 18
# speedup vs baseline: 1.0356x; 1.0321x over previous
"""OTAM (5-way 5-shot video few-shot) kernel for Trainium2, 8 NeuronCores.

Self-contained: kernel(**inputs) takes full inputs, shards 512 queries over
8 cores (64 each), runs a Bass/Tile kernel per core, gathers class means.

v4 design (v0 328us, v1 175us, v2 142us, v3 131us):
 - bf16 matmuls + transposes; norms ACT Square + Quake rsqrt (DVE); q stays
   unnormalized, 10/||q|| folded into the exp scale AP (norm branch runs
   parallel to the transpose branch)
 - support tensor reordered HOST-side to (tau, s) row order so the staged
   cost tile is [t][tau][s] -> every DP operand is contiguous
 - software-pipelined PE stream: transposes of group g+1 are emitted before
   the matmuls of group g so the PE never waits on the scalar qt copy
 - cost tile staged twice (both DP dirs) via 800B-packet DMAs; DP on 128
   partitions, W layout [l][s], bf16, 2 renorms (int16 exponent tricks)
"""
import sys
sys.path.insert(0, "/opt/trn_rl_repo")
import numpy as np
from contextlib import ExitStack

import concourse.bacc as bacc
import concourse.tile as tile
from concourse import mybir
from concourse.masks import make_identity

F32 = mybir.dt.float32
BF16 = mybir.dt.bfloat16
I32 = mybir.dt.int32
I16 = mybir.dt.int16
AF = mybir.ActivationFunctionType
ALU = mybir.AluOpType
LN2 = float(np.log(2.0))

NS, T, D = 25, 16, 2048
NQ_CORE = 64
G = 8                        # query groups of 128 rows (8 queries) each
NSTAU = NS * T               # 400
KCH = D // 128               # 16
SROWS = [128, 128, 128, 16]


def quake_rsqrt(nc, pool, x_f32, nrow, tag, scale=1.0):
    """y ~= scale/sqrt(x) on [nrow,1] f32 (DVE only; 1 Newton iteration)."""
    y = pool.tile([128, 1], F32, tag=tag + "_y")
    t = pool.tile([128, 1], F32, tag=tag + "_t")
    yi = y.bitcast(I32)
    nc.vector.tensor_scalar(yi[:nrow], x_f32[:nrow].bitcast(I32), 1, None,
                            op0=ALU.logical_shift_right)
    nc.vector.tensor_scalar(yi[:nrow], yi[:nrow], 0x5F3759DF, -1,
                            op0=ALU.subtract, op1=ALU.mult)
    nc.vector.tensor_tensor(t[:nrow], y[:nrow], y[:nrow], op=ALU.mult)
    nc.vector.tensor_tensor(t[:nrow], t[:nrow], x_f32[:nrow], op=ALU.mult)
    nc.vector.tensor_scalar(t[:nrow], t[:nrow], -0.5 * scale, 1.5 * scale,
                            op0=ALU.mult, op1=ALU.add)
    nc.vector.tensor_tensor(y[:nrow], y[:nrow], t[:nrow], op=ALU.mult)
    return y


def build_core_kernel():
    nc = bacc.Bacc("TRN2", target_bir_lowering=False, debug=False)

    q_d = nc.dram_tensor("q", [NQ_CORE * T, D], F32, kind="ExternalInput").ap()
    # s rows are HOST-reordered to (tau, s): row index = tau*NS + s
    s_d = nc.dram_tensor("s", [NSTAU, D], F32, kind="ExternalInput").ap()
    out_d = nc.dram_tensor("out", [128, NS], F32, kind="ExternalOutput").ap()

    with tile.TileContext(nc) as tc, ExitStack() as ctx:
        const = ctx.enter_context(tc.tile_pool(name="const", bufs=1))
        eye_b = const.tile([128, 128], BF16, tag="eye_b")
        make_identity(nc, eye_b[:])
        bias_m10 = const.tile([128, 1], F32, tag="bias_m10")
        nc.vector.memset(bias_m10[:], -10.0)

        # normalized+transposed support: st_b[p=d%128][k=d//128][col=(tau,s)]
        stp = ctx.enter_context(tc.tile_pool(name="stp", bufs=1))
        st_b = stp.tile([128, KCH, NSTAU], BF16, tag="st_b")

        nsc = ctx.enter_context(tc.tile_pool(name="nsc", bufs=1))
        dmp = ctx.enter_context(tc.tile_pool(name="dmp", bufs=1))
        dump = dmp.tile([128, D], BF16, tag="dump")

        # ---------------- pools ----------------
        cp = ctx.enter_context(tc.tile_pool(name="cp", bufs=1))
        # c_t[p][t][tau][s] bf16: partitions q and 64+q hold query q's costs
        c_t = cp.tile([128, T, T, NS], BF16, tag="c_t")

        qldp = ctx.enter_context(tc.tile_pool(name="qldp", bufs=3))
        qbfp = ctx.enter_context(tc.tile_pool(name="qbfp", bufs=3))
        qtp = ctx.enter_context(tc.tile_pool(name="qtp", bufs=3))
        t1p = ctx.enter_context(tc.tile_pool(name="t1p", bufs=2))
        ptr = ctx.enter_context(tc.tile_pool(name="ptr", bufs=2, space="PSUM"))
        pmm = ctx.enter_context(tc.tile_pool(name="pmm", bufs=2, space="PSUM"))

        # ---------------- Q phase: software-pipelined over 8 groups -------
        def q_load(g):
            qraw = qldp.tile([128, D], F32, tag="qraw")
            nc.sync.dma_start(out=qraw[:], in_=q_d[128 * g:128 * (g + 1), :])
            return qraw

        def q_norm(qbf, g):
            n2 = nsc.tile([128, 1], F32, tag="qn2")
            nc.scalar.activation(dump[:], qbf[:], AF.Square,
                                 accum_out=n2[:])
            return quake_rsqrt(nc, nsc, n2, 128, "qrs", scale=10.0)

        def q_cast(qraw):
            qbf = qbfp.tile([128, D], BF16, tag="qbf")
            nc.vector.tensor_copy(qbf[:], qraw[:])
            return qbf

        def q_transpose(qbf):
            pt = ptr.tile([128, D], BF16, tag="pt")
            for k in range(KCH):
                nc.tensor.transpose(pt[:, 128 * k:128 * (k + 1)],
                                    qbf[:, 128 * k:128 * (k + 1)], eye_b[:])
            qt = qtp.tile([128, KCH, 128], BF16, tag="qt")
            half = KCH // 2
            ptv = pt[:].rearrange("p (k f) -> p k f", k=KCH)
            nc.scalar.copy(qt[:, 0:half], ptv[:, 0:half])
            nc.vector.tensor_copy(qt[:, half:KCH], ptv[:, half:KCH])
            return qt

        def q_mm_exp_stage(g, qt, rq10):
            mm = pmm.tile([128, NSTAU], F32, tag="mm")
            for k in range(KCH):
                nc.tensor.matmul(mm[:], qt[:, k, :], st_b[:, k, :],
                                 start=(k == 0), stop=(k == KCH - 1))
            t1 = t1p.tile([128, NSTAU], BF16, tag="t1")
            nc.scalar.activation(t1[:], mm[:], AF.Exp, bias=bias_m10[:],
                                 scale=rq10[:])
            nc.scalar.dma_start(out=c_t[8 * g:8 * (g + 1), :, :, :], in_=t1[:])
            nc.scalar.dma_start(out=c_t[64 + 8 * g:64 + 8 * (g + 1), :, :, :],
                                in_=t1[:])


        # ---------------- fronts for groups 0 and 1 (before S phase) ----
        qraws = [None] * G
        qts = [None] * G
        rqs = [None] * G
        qraws[0] = q_load(0)
        qraws[1] = q_load(1)

        # ---------------- S phase ----------------
        with tc.tile_pool(name="sraw", bufs=1) as sraw, \
             tc.tile_pool(name="spsum", bufs=2, space="PSUM") as spsum:
            snorm = []
            for i, nrow in enumerate(SROWS):
                t_ = sraw.tile([128, D], F32, tag=f"sraw{i}")
                nc.sync.dma_start(out=t_[:nrow],
                                  in_=s_d[128 * i:128 * i + nrow, :])
                sb = sraw.tile([128, D], BF16, tag=f"sbf{i}")
                nc.vector.tensor_copy(sb[:nrow], t_[:nrow])
                snorm.append((sb, nrow))
            n2s = []
            for i, (sb, nrow) in enumerate(snorm):
                n2 = nsc.tile([128, 1], F32, tag=f"sn2_{i}")
                nc.scalar.activation(dump[:nrow], sb[:nrow], AF.Square,
                                     accum_out=n2[:nrow])
                n2s.append(n2)
            for i, (sb, nrow) in enumerate(snorm):
                rs = quake_rsqrt(nc, nsc, n2s[i], nrow, f"srs{i}")
                nc.vector.tensor_scalar(sb[:nrow], sb[:nrow], rs[:nrow], None,
                                        op0=ALU.mult)
            for k in range(KCH):
                ps = spsum.tile([128, 512], BF16, tag="sps")
                for i, (sb, nrow) in enumerate(snorm):
                    nc.tensor.transpose(ps[:, 128 * i:128 * i + nrow],
                                        sb[:nrow, 128 * k:128 * (k + 1)],
                                        eye_b[:nrow, :nrow])
                nc.scalar.copy(st_b[:, k, :], ps[:, 0:NSTAU])

        qbf0 = q_cast(qraws[0])
        rqs[0] = q_norm(qbf0, 0)
        qts[0] = q_transpose(qbf0)
        qbf1 = q_cast(qraws[1])
        rqs[1] = q_norm(qbf1, 1)
        qts[1] = q_transpose(qbf1)

        # modulo software pipeline with distance 2: fronts for groups 0/1
        # were already emitted before the S phase; the loop emits front(g+2)
        # then back(g).
        for g in range(G):
            if g + 2 < G:
                qraws[g + 2] = q_load(g + 2)
                qbf_n = q_cast(qraws[g + 2])
                rqs[g + 2] = q_norm(qbf_n, g + 2)
                qts[g + 2] = q_transpose(qbf_n)
            q_mm_exp_stage(g, qts[g], rqs[g])

        # ---------------- DP phase (exp domain) ----------------
        # partition q: dir "b" (rows l = support frame tau, cols = t)
        # partition 64+q: dir "a" (rows l = query frame t, cols = tau)
        # W layout [l][s] so W slices and dir-b cost reads are contiguous
        dpp = ctx.enter_context(tc.tile_pool(name="dpp", bufs=1))
        w_t = dpp.tile([128, T + 1, NS], BF16, tag="w_t")
        nc.vector.memset(w_t[:], 2.0)
        nc.vector.memset(w_t[:, 0:1, :], 1.0)
        o_t = dpp.tile([128, NS], F32, tag="o_t")
        nc.vector.memset(o_t[:], 0.0)
        scratch = dpp.tile([128, T, NS], BF16, tag="scratch")
        kmax = dpp.tile([128, NS], BF16, tag="kmax")
        masked = dpp.tile([128, NS], I16, tag="masked")
        krec = dpp.tile([128, NS], I16, tag="krec")
        ef = dpp.tile([128, NS], F32, tag="ef")
        otmp = dpp.tile([128, NS], F32, tag="otmp")

        def renorm(a):
            wsl = w_t[:, a:T + 1, :]
            nc.vector.tensor_reduce(kmax[:], wsl.rearrange("p l s -> p s l"),
                                    axis=mybir.AxisListType.X, op=ALU.max)
            nc.vector.tensor_scalar(masked[:], kmax[:].bitcast(I16),
                                    0x7F80, None, op0=ALU.bitwise_and)
            nc.vector.tensor_scalar(krec[:], masked[:], 0x7F00, -1,
                                    op0=ALU.subtract, op1=ALU.mult)
            nc.vector.tensor_copy(ef[:], masked[:])
            nc.vector.tensor_scalar(otmp[:], ef[:], LN2 / (1 << 7),
                                    -127.0 * LN2, op0=ALU.mult, op1=ALU.add)
            nc.vector.tensor_tensor(o_t[:], o_t[:], otmp[:], op=ALU.add)
            nc.vector.tensor_tensor(
                wsl, wsl,
                krec[:].bitcast(BF16).unsqueeze(-1)
                    .broadcast_to((128, NS, T + 1 - a))
                    .rearrange("p s l -> p l s"),
                op=ALU.mult)

        for m in range(2, T + 3):           # m = 2..18
            j0 = max(1, m - 2)
            wm = (T + 1) - j0
            if m == T + 2:                  # last: dup, cost=1, l=T only
                nc.vector.scalar_tensor_tensor(
                    w_t[:, T:T + 1, :], w_t[:, T:T + 1, :], 2.0,
                    w_t[:, T - 1:T, :], op0=ALU.mult, op1=ALU.add)
                break
            wact = w_t[:, j0:T + 1, :]
            wsh = w_t[:, j0 - 1:T, :]
            tmp = scratch[:, 0:wm, :]
            if m == 2:
                nc.vector.scalar_tensor_tensor(tmp, wact, 2.0, wsh,
                                               op0=ALU.mult, op1=ALU.add)
            else:
                nc.vector.tensor_tensor(tmp, wact, wsh, op=ALU.add)
            # dir b on partitions 0:64 (contiguous cost read)
            cb = c_t[0:64, m - 2, j0 - 1:j0 - 1 + wm, :]
            nc.vector.tensor_tensor(w_t[0:64, j0:T + 1, :], tmp[0:64], cb,
                                    op=ALU.mult)
            # dir a on partitions 64:128 (l strided, s contiguous)
            ca = c_t[64:128, j0 - 1:j0 - 1 + wm, m - 2, :]
            nc.vector.tensor_tensor(w_t[64:128, j0:T + 1, :], tmp[64:128], ca,
                                    op=ALU.mult)
            if m in (9, 16):
                renorm(m - 2)

        lnw = dpp.tile([128, NS], F32, tag="lnw")
        nc.scalar.activation(lnw[:], w_t[:, T, :], AF.Ln)
        ans = dpp.tile([128, NS], F32, tag="ans")
        nc.vector.tensor_tensor(ans[:], lnw[:], o_t[:], op=ALU.add)
        nc.sync.dma_start(out=out_d, in_=ans[:])

    nc.compile()
    return nc


_NC_CACHE = {}


def _get_nc():
    if "nc" not in _NC_CACHE:
        _NC_CACHE["nc"] = build_core_kernel()
    return _NC_CACHE["nc"]


def kernel(support_features, target_features, support_labels):
    out, _ = host_kernel(support_features, target_features, support_labels,
                         nc=_get_nc())
    return out


def host_kernel(support_features, target_features, support_labels, nc=None,
                run_hw=True, trace=False):
    n_support, T_, d = support_features.shape
    nq = target_features.shape[0]
    assert (n_support, T_, d) == (NS, T, D) and nq == 512
    if nc is None:
        nc = build_core_kernel()
    # reorder support rows to (tau, s) so mm columns come out (tau, s)-major
    s_flat = np.ascontiguousarray(
        np.asarray(support_features).transpose(1, 0, 2).reshape(NSTAU, D))
    in_maps = []
    for c in range(8):
        qs = target_features[64 * c:64 * (c + 1)].reshape(NQ_CORE * T, D)
        in_maps.append({"q": np.ascontiguousarray(qs), "s": s_flat})
    from concourse.bass_utils import run_bass_kernel_spmd
    res = run_bass_kernel_spmd(nc, in_maps, list(range(8)), trace=trace)
    outs = [np.asarray(r["out"]) for r in res.results]
    # partition q = dir b, partition 64+q = dir a; dists = -0.1*(a+b)
    dists = np.concatenate(
        [-0.1 * (o[0:64] + o[64:128]) for o in outs], axis=0)
    onehot = (np.asarray(support_labels)[:, None]
              == np.arange(5)[None, :]).astype(np.float32)
    class_dists = (dists.astype(np.float32) @ onehot) / onehot.sum(axis=0)
    return class_dists.astype(np.float32), res


# revision 19
# speedup vs baseline: 1.0631x; 1.0265x over previous
"""OTAM (5-way 5-shot video few-shot) kernel for Trainium2, 8 NeuronCores.

Self-contained: kernel(**inputs) takes full inputs, shards 512 queries over
8 cores (64 each), runs a Bass/Tile kernel per core, gathers class means.

v4 design (v0 328us, v1 175us, v2 142us, v3 131us):
 - bf16 matmuls + transposes; norms ACT Square + Quake rsqrt (DVE); q stays
   unnormalized, 10/||q|| folded into the exp scale AP (norm branch runs
   parallel to the transpose branch)
 - support tensor reordered HOST-side to (tau, s) row order so the staged
   cost tile is [t][tau][s] -> every DP operand is contiguous
 - software-pipelined PE stream: transposes of group g+1 are emitted before
   the matmuls of group g so the PE never waits on the scalar qt copy
 - cost tile staged twice (both DP dirs) via 800B-packet DMAs; DP on 128
   partitions, W layout [l][s], bf16, 2 renorms (int16 exponent tricks)
"""
import sys
sys.path.insert(0, "/opt/trn_rl_repo")
import numpy as np
from contextlib import ExitStack

import concourse.bacc as bacc
import concourse.tile as tile
from concourse import mybir
from concourse.masks import make_identity

F32 = mybir.dt.float32
BF16 = mybir.dt.bfloat16
I32 = mybir.dt.int32
I16 = mybir.dt.int16
AF = mybir.ActivationFunctionType
ALU = mybir.AluOpType
LN2 = float(np.log(2.0))

NS, T, D = 25, 16, 2048
NQ_CORE = 64
G = 8                        # query groups of 128 rows (8 queries) each
NSTAU = NS * T               # 400
KCH = D // 128               # 16
SROWS = [128, 128, 128, 16]


def quake_rsqrt(nc, pool, x_f32, nrow, tag, scale=1.0):
    """y ~= scale/sqrt(x) on [nrow,1] f32 (DVE only; 1 Newton iteration)."""
    y = pool.tile([128, 1], F32, tag=tag + "_y")
    t = pool.tile([128, 1], F32, tag=tag + "_t")
    yi = y.bitcast(I32)
    nc.vector.tensor_scalar(yi[:nrow], x_f32[:nrow].bitcast(I32), 1, None,
                            op0=ALU.logical_shift_right)
    nc.vector.tensor_scalar(yi[:nrow], yi[:nrow], 0x5F3759DF, -1,
                            op0=ALU.subtract, op1=ALU.mult)
    nc.vector.tensor_tensor(t[:nrow], y[:nrow], y[:nrow], op=ALU.mult)
    nc.vector.tensor_tensor(t[:nrow], t[:nrow], x_f32[:nrow], op=ALU.mult)
    nc.vector.tensor_scalar(t[:nrow], t[:nrow], -0.5 * scale, 1.5 * scale,
                            op0=ALU.mult, op1=ALU.add)
    nc.vector.tensor_tensor(y[:nrow], y[:nrow], t[:nrow], op=ALU.mult)
    return y


def build_core_kernel():
    nc = bacc.Bacc("TRN2", target_bir_lowering=False, debug=False)

    q_d = nc.dram_tensor("q", [NQ_CORE * T, D], F32, kind="ExternalInput").ap()
    # s rows are HOST-reordered to (tau, s): row index = tau*NS + s
    s_d = nc.dram_tensor("s", [NSTAU, D], F32, kind="ExternalInput").ap()
    out_d = nc.dram_tensor("out", [128, NS], F32, kind="ExternalOutput").ap()

    with tile.TileContext(nc) as tc, ExitStack() as ctx:
        const = ctx.enter_context(tc.tile_pool(name="const", bufs=1))
        eye_b = const.tile([128, 128], BF16, tag="eye_b")
        make_identity(nc, eye_b[:])
        bias_m10 = const.tile([128, 1], F32, tag="bias_m10")
        nc.vector.memset(bias_m10[:], -10.0)

        # normalized+transposed support: st_b[p=d%128][k=d//128][col=(tau,s)]
        stp = ctx.enter_context(tc.tile_pool(name="stp", bufs=1))
        st_b = stp.tile([128, KCH, NSTAU], BF16, tag="st_b")

        nsc = ctx.enter_context(tc.tile_pool(name="nsc", bufs=1))
        dmp = ctx.enter_context(tc.tile_pool(name="dmp", bufs=1))
        dump = dmp.tile([128, D], BF16, tag="dump")

        # ---------------- pools ----------------
        cp = ctx.enter_context(tc.tile_pool(name="cp", bufs=1))
        # c_t[p][t][tau][s] bf16: partitions q and 64+q hold query q's costs
        c_t = cp.tile([128, T, T, NS], BF16, tag="c_t")

        qldp = ctx.enter_context(tc.tile_pool(name="qldp", bufs=3))
        qbfp = ctx.enter_context(tc.tile_pool(name="qbfp", bufs=3))
        qtp = ctx.enter_context(tc.tile_pool(name="qtp", bufs=3))
        t1p = ctx.enter_context(tc.tile_pool(name="t1p", bufs=2))
        ptr = ctx.enter_context(tc.tile_pool(name="ptr", bufs=2, space="PSUM"))
        pmm = ctx.enter_context(tc.tile_pool(name="pmm", bufs=2, space="PSUM"))

        # ---------------- S phase ----------------
        with tc.tile_pool(name="sraw", bufs=1) as sraw, \
             tc.tile_pool(name="spsum", bufs=2, space="PSUM") as spsum:
            snorm = []
            for i, nrow in enumerate(SROWS):
                t_ = sraw.tile([128, D], F32, tag=f"sraw{i}")
                nc.sync.dma_start(out=t_[:nrow],
                                  in_=s_d[128 * i:128 * i + nrow, :])
                sb = sraw.tile([128, D], BF16, tag=f"sbf{i}")
                nc.vector.tensor_copy(sb[:nrow], t_[:nrow])
                snorm.append((sb, nrow))
            n2s = []
            for i, (sb, nrow) in enumerate(snorm):
                n2 = nsc.tile([128, 1], F32, tag=f"sn2_{i}")
                nc.scalar.activation(dump[:nrow], sb[:nrow], AF.Square,
                                     accum_out=n2[:nrow])
                n2s.append(n2)
            for i, (sb, nrow) in enumerate(snorm):
                rs = quake_rsqrt(nc, nsc, n2s[i], nrow, f"srs{i}")
                nc.vector.tensor_scalar(sb[:nrow], sb[:nrow], rs[:nrow], None,
                                        op0=ALU.mult)
            for k in range(KCH):
                ps = spsum.tile([128, 512], BF16, tag="sps")
                for i, (sb, nrow) in enumerate(snorm):
                    nc.tensor.transpose(ps[:, 128 * i:128 * i + nrow],
                                        sb[:nrow, 128 * k:128 * (k + 1)],
                                        eye_b[:nrow, :nrow])
                nc.scalar.copy(st_b[:, k, :], ps[:, 0:NSTAU])

        # ---------------- Q phase: software-pipelined over 8 groups -------
        def q_load(g):
            qraw = qldp.tile([128, D], F32, tag="qraw")
            nc.sync.dma_start(out=qraw[:], in_=q_d[128 * g:128 * (g + 1), :])
            return qraw

        def q_norm(qbf):
            n2 = nsc.tile([128, 1], F32, tag="qn2")
            nc.scalar.activation(dump[:], qbf[:], AF.Square, accum_out=n2[:])
            return quake_rsqrt(nc, nsc, n2, 128, "qrs", scale=10.0)

        def q_cast(qraw):
            qbf = qbfp.tile([128, D], BF16, tag="qbf")
            nc.vector.tensor_copy(qbf[:], qraw[:])
            return qbf

        def q_transpose(qbf):
            pt = ptr.tile([128, D], BF16, tag="pt")
            for k in range(KCH):
                nc.tensor.transpose(pt[:, 128 * k:128 * (k + 1)],
                                    qbf[:, 128 * k:128 * (k + 1)], eye_b[:])
            qt = qtp.tile([128, KCH, 128], BF16, tag="qt")
            half = KCH // 2
            ptv = pt[:].rearrange("p (k f) -> p k f", k=KCH)
            nc.scalar.copy(qt[:, 0:half], ptv[:, 0:half])
            nc.vector.tensor_copy(qt[:, half:KCH], ptv[:, half:KCH])
            return qt

        def q_mm_exp_stage(g, qt, rq10):
            mm = pmm.tile([128, NSTAU], F32, tag="mm")
            for k in range(KCH):
                nc.tensor.matmul(mm[:], qt[:, k, :], st_b[:, k, :],
                                 start=(k == 0), stop=(k == KCH - 1))
            t1 = t1p.tile([128, NSTAU], BF16, tag="t1")
            nc.scalar.activation(t1[:], mm[:], AF.Exp, bias=bias_m10[:],
                                 scale=rq10[:])
            nc.scalar.dma_start(out=c_t[8 * g:8 * (g + 1), :, :, :], in_=t1[:])
            nc.scalar.dma_start(out=c_t[64 + 8 * g:64 + 8 * (g + 1), :, :, :],
                                in_=t1[:])

        # modulo software pipeline: front(g) = load/norm/cast/transpose,
        # back(g) = mm/exp/stage.  Emit front(g+1) before back(g).
        qraws = [None] * G
        qraws[0] = q_load(0)
        if G > 1:
            qraws[1] = q_load(1)
        qbf0 = q_cast(qraws[0])
        rq_prev = q_norm(qbf0)
        qt_prev = q_transpose(qbf0)
        for g in range(G):
            qt_cur, rq_cur = qt_prev, rq_prev
            if g + 1 < G:
                if g + 2 < G:
                    qraws[g + 2] = q_load(g + 2)
                qbf_n = q_cast(qraws[g + 1])
                rq_prev = q_norm(qbf_n)
                qt_prev = q_transpose(qbf_n)
            q_mm_exp_stage(g, qt_cur, rq_cur)

        # ---------------- DP phase (exp domain) ----------------
        # partition q: dir "b" (rows l = support frame tau, cols = t)
        # partition 64+q: dir "a" (rows l = query frame t, cols = tau)
        # W layout [l][s] so W slices and dir-b cost reads are contiguous
        dpp = ctx.enter_context(tc.tile_pool(name="dpp", bufs=1))
        w_t = dpp.tile([128, T + 1, NS], BF16, tag="w_t")
        nc.vector.memset(w_t[:], 2.0)
        nc.vector.memset(w_t[:, 0:1, :], 1.0)
        o_t = dpp.tile([128, NS], F32, tag="o_t")
        nc.vector.memset(o_t[:], 0.0)
        scratch = dpp.tile([128, T, NS], BF16, tag="scratch")
        kmax = dpp.tile([128, NS], BF16, tag="kmax")
        masked = dpp.tile([128, NS], I16, tag="masked")
        krec = dpp.tile([128, NS], I16, tag="krec")
        ef = dpp.tile([128, NS], F32, tag="ef")
        otmp = dpp.tile([128, NS], F32, tag="otmp")

        def renorm(a):
            wsl = w_t[:, a:T + 1, :]
            nc.vector.tensor_reduce(kmax[:], wsl.rearrange("p l s -> p s l"),
                                    axis=mybir.AxisListType.X, op=ALU.max)
            nc.vector.tensor_scalar(masked[:], kmax[:].bitcast(I16),
                                    0x7F80, None, op0=ALU.bitwise_and)
            nc.vector.tensor_scalar(krec[:], masked[:], 0x7F00, -1,
                                    op0=ALU.subtract, op1=ALU.mult)
            nc.vector.tensor_copy(ef[:], masked[:])
            nc.vector.tensor_scalar(otmp[:], ef[:], LN2 / (1 << 7),
                                    -127.0 * LN2, op0=ALU.mult, op1=ALU.add)
            nc.vector.tensor_tensor(o_t[:], o_t[:], otmp[:], op=ALU.add)
            nc.vector.tensor_tensor(
                wsl, wsl,
                krec[:].bitcast(BF16).unsqueeze(-1)
                    .broadcast_to((128, NS, T + 1 - a))
                    .rearrange("p s l -> p l s"),
                op=ALU.mult)

        for m in range(2, T + 3):           # m = 2..18
            j0 = max(1, m - 2)
            wm = (T + 1) - j0
            if m == T + 2:                  # last: dup, cost=1, l=T only
                nc.vector.scalar_tensor_tensor(
                    w_t[:, T:T + 1, :], w_t[:, T:T + 1, :], 2.0,
                    w_t[:, T - 1:T, :], op0=ALU.mult, op1=ALU.add)
                break
            wact = w_t[:, j0:T + 1, :]
            wsh = w_t[:, j0 - 1:T, :]
            tmp = scratch[:, 0:wm, :]
            if m == 2:
                nc.vector.scalar_tensor_tensor(tmp, wact, 2.0, wsh,
                                               op0=ALU.mult, op1=ALU.add)
            else:
                nc.vector.tensor_tensor(tmp, wact, wsh, op=ALU.add)
            # dir b on partitions 0:64 (contiguous cost read)
            cb = c_t[0:64, m - 2, j0 - 1:j0 - 1 + wm, :]
            nc.vector.tensor_tensor(w_t[0:64, j0:T + 1, :], tmp[0:64], cb,
                                    op=ALU.mult)
            # dir a on partitions 64:128 (l strided, s contiguous)
            ca = c_t[64:128, j0 - 1:j0 - 1 + wm, m - 2, :]
            nc.vector.tensor_tensor(w_t[64:128, j0:T + 1, :], tmp[64:128], ca,
                                    op=ALU.mult)
            if m in (9, 16):
                renorm(m - 2)

        lnw = dpp.tile([128, NS], F32, tag="lnw")
        nc.scalar.activation(lnw[:], w_t[:, T, :], AF.Ln)
        ans = dpp.tile([128, NS], F32, tag="ans")
        nc.vector.tensor_tensor(ans[:], lnw[:], o_t[:], op=ALU.add)
        nc.sync.dma_start(out=out_d, in_=ans[:])

    nc.compile()
    return nc


_NC_CACHE = {}


def _get_nc():
    if "nc" not in _NC_CACHE:
        _NC_CACHE["nc"] = build_core_kernel()
    return _NC_CACHE["nc"]


def kernel(support_features, target_features, support_labels):
    out, _ = host_kernel(support_features, target_features, support_labels,
                         nc=_get_nc())
    return out


def host_kernel(support_features, target_features, support_labels, nc=None,
                run_hw=True, trace=False):
    n_support, T_, d = support_features.shape
    nq = target_features.shape[0]
    assert (n_support, T_, d) == (NS, T, D) and nq == 512
    if nc is None:
        nc = build_core_kernel()
    # reorder support rows to (tau, s) so mm columns come out (tau, s)-major
    s_flat = np.ascontiguousarray(
        np.asarray(support_features).transpose(1, 0, 2).reshape(NSTAU, D))
    in_maps = []
    for c in range(8):
        qs = target_features[64 * c:64 * (c + 1)].reshape(NQ_CORE * T, D)
        in_maps.append({"q": np.ascontiguousarray(qs), "s": s_flat})
    from concourse.bass_utils import run_bass_kernel_spmd
    res = run_bass_kernel_spmd(nc, in_maps, list(range(8)), trace=trace)
    outs = [np.asarray(r["out"]) for r in res.results]
    # partition q = dir b, partition 64+q = dir a; dists = -0.1*(a+b)
    dists = np.concatenate(
        [-0.1 * (o[0:64] + o[64:128]) for o in outs], axis=0)
    onehot = (np.asarray(support_labels)[:, None]
              == np.arange(5)[None, :]).astype(np.float32)
    class_dists = (dists.astype(np.float32) @ onehot) / onehot.sum(axis=0)
    return class_dists.astype(np.float32), res
